# revision 18
# baseline (speedup 1.0000x reference)
"""Trainium2 Bass kernel for the NCE cosine-similarity loss.

Problem: x [65536, 1024] f32 viewed as 1024 batches x 64 rows (1 orig, 8 pos,
55 neg). Per batch: cos(orig,pos_i) and cos(pos_i,neg_j), logits/0.1,
loss = logsumexp([cp, cn_*]) - cp, mean over all (batch, pos).

Strategy (8 NeuronCores, data-parallel over batches, 128 batches/core):
 - Host staging: shard over cores, pre-cast to fp8-e4m3 (dots tolerate it;
   final rel err ~2e-5), pre-TRANSPOSE so the
   contraction dim d lands on SBUF partitions (kills all PE transposes that
   dominated the previous version), and reorder each pair's 128 rows to
   [posA(8) posB(8) origA origB negA(55) negB(55)].
 - Per pair (2 batches = 128 rows): full 128x128 Gram via 8 accumulating
   fp8 matmuls (K=128 d-chunks, stream N=128). Diagonal = squared norms
   (batched DVE multiply+reduce vs identity). Pos rows 0..15 of each
   pair's Gram are the only cosines needed.
 - Per group of 8 pairs, everything else is batched on 128 partitions:
   inv = sqrt(10)/n (folds 1/tau; ACT Ln+Exp), a masked scale matrix
   bcig[16j+i, r] = inv_j[i]*inv_j[r]*mask (built with 3 tiny PE matmuls),
   one DVE multiply -> logits, one ACT Exp with accum_out -> row sums
   (excluded logits are scaled to 0 so each contributes exp(0)=1; the final
   log subtracts the 72 ones via its bias), one fused reduce -> l0.
 - loss rows stage into [128, 8]; final Ln(sum-72)+accum and a reduce give
   [128, 2] per core; host combines 8 cores and divides by 8192.
"""

import sys

if "/opt/trn_rl_repo" not in sys.path:
    sys.path.insert(0, "/opt/trn_rl_repo")

import numpy as np

N_CORES = 8
ROWS_PER_CORE = 8192          # 128 batches x 64 rows
D = 1024
N_GROUPS = 8                  # groups of 8 pairs per core
N_PAIRS = 64                  # 2 batches per pair
NB = 64                       # rows per batch
NPOS = 8

_CACHE = {}


def _build(loop_n=0, fp8=False, stage=3, dmaeng="sync"):
    import concourse.bacc as bacc
    import concourse.mybir as mybir
    import concourse.tile as tile

    dt = mybir.dt
    AF = mybir.ActivationFunctionType
    ALU = mybir.AluOpType
    xdt = dt.float8e4 if fp8 else dt.bfloat16

    nc = bacc.Bacc("TRN2", target_bir_lowering=False, debug=False, num_devices=N_CORES)
    x = nc.dram_tensor("x", [N_GROUPS, 128, 8 * D], xdt, kind="ExternalInput")
    identb_d = nc.dram_tensor("identb", [128, 128], dt.bfloat16, kind="ExternalInput")
    maskab_d = nc.dram_tensor("maskab", [16, 128], dt.bfloat16, kind="ExternalInput")
    pattb_d = nc.dram_tensor("pattb", [16, 256], dt.bfloat16, kind="ExternalInput")
    l0mask_d = nc.dram_tensor("l0mask", [128, 2], dt.bfloat16, kind="ExternalInput")
    neg72_d = nc.dram_tensor("neg72", [128, 1], dt.float32, kind="ExternalInput")
    out_d = nc.dram_tensor("out", [128, 2], dt.float32, kind="ExternalOutput")

    with tile.TileContext(nc) as tc:
        from contextlib import ExitStack, nullcontext

        with ExitStack() as ctx:
            cpool = ctx.enter_context(tc.tile_pool(name="consts", bufs=1))
            rowp = ctx.enter_context(tc.tile_pool(name="row", bufs=8))
            gramp = ctx.enter_context(tc.tile_pool(name="gram", bufs=4, space="PSUM"))
            bcpsp = ctx.enter_context(tc.tile_pool(name="bcps", bufs=2, space="PSUM"))
            itpsp = ctx.enter_context(tc.tile_pool(name="itps", bufs=1, space="PSUM"))
            sb = ctx.enter_context(tc.tile_pool(name="sb", bufs=3))
            scrp = ctx.enter_context(tc.tile_pool(name="scr", bufs=3))
            stg = ctx.enter_context(tc.tile_pool(name="stg", bufs=1))

            identb = cpool.tile([128, 128], dt.bfloat16)
            nc.sync.dma_start(out=identb[:], in_=identb_d[:])
            maskab = cpool.tile([16, 128], dt.bfloat16)
            nc.sync.dma_start(out=maskab[:], in_=maskab_d[:])
            pattb = cpool.tile([16, 256], dt.bfloat16)
            nc.sync.dma_start(out=pattb[:], in_=pattb_d[:])
            l0mask = cpool.tile([128, 2], dt.bfloat16)
            nc.sync.dma_start(out=l0mask[:], in_=l0mask_d[:])
            neg72 = cpool.tile([128, 1], dt.float32)
            nc.sync.dma_start(out=neg72[:], in_=neg72_d[:])

            sums_stage = stg.tile([128, 2 * N_GROUPS], dt.float32, tag="sums")
            l0_stage = stg.tile([128, 2 * N_GROUPS], dt.float32, tag="l0s")

            def phase_a(g, mid=None):
                # two half-group tiles: quad-0 grams start after half the
                # bytes land, and the second DMA overlaps quad-0 compute
                rts = []
                for h in range(2):
                    rt = rowp.tile([128, 4, 8, 128], xdt, tag="rt")
                    if dmaeng == "sync":
                        nc.sync.dma_start(out=rt[:], in_=x[g, :, 4096 * h : 4096 * h + 4096])
                    else:
                        nc.gpsimd.dma_start(out=rt[:], in_=x[g, :, 4096 * h : 4096 * h + 4096])
                    rts.append(rt)
                if stage < 1:
                    return None, None

                n2g = sb.tile([128, 8], dt.float32, tag="n2g")
                # posG[32*(j%4)+u, 128*(j//4)+r] = G_j[u, r]; only u<16 (pos
                # rows) are used downstream, u in 16..32 ride along so every
                # partition start stays 32-aligned (HW AP constraint).
                posG = sb.tile([128, 2, 128], dt.bfloat16, tag="posG")
                for e in range(2):
                    if e == 1 and mid is not None:
                        # emit the previous group's tail here so its small
                        # PE/DVE/ACT chain only waits behind half a group of
                        # gram matmuls in the FIFO engine queues
                        phase_b(mid[0], *mid[1])
                    gps = gramp.tile([128, 4, 128], dt.float32, tag="gram")
                    for jj in range(4):
                        for c in range(8):
                            nc.tensor.matmul(
                                gps[:, jj, :],
                                rts[e][:, jj, c, :],
                                rts[e][:, jj, c, :],
                                start=(c == 0),
                                stop=(c == 7),
                            )
                        if stage >= 2:
                            nc.scalar.copy(
                                posG[32 * jj : 32 * jj + 32, e, :], gps[0:32, jj, :]
                            )
                    if stage < 2:
                        continue
                    # batched diag: one mult + one reduce per 4 pairs
                    dscr = scrp.tile([128, 4, 128], dt.float16, tag="dscr")
                    nc.vector.tensor_mul(
                        dscr[:],
                        gps[:],
                        identb.rearrange("p (o b) -> p o b", o=1).broadcast_to(
                            [128, 4, 128]
                        ),
                    )
                    nc.vector.reduce_sum(
                        n2g[:, 4 * e : 4 * e + 4], dscr[:], axis=mybir.AxisListType.X
                    )
                return n2g, posG

            def phase_b(g, n2g, posG):
                # inv = sqrt(10)/n = exp(-0.5*ln(0.1*n2)); Ln/Exp share a table set
                lnn2 = sb.tile([128, 8], dt.float32, tag="lnn2")
                nc.scalar.activation(lnn2[:], n2g[:], AF.Ln, scale=0.1)
                invf = sb.tile([128, 8], dt.float32, tag="invf")
                nc.scalar.activation(invf[:], lnn2[:], AF.Exp, scale=-0.5)
                invb2 = sb.tile([128, 16], dt.bfloat16, tag="invb2")
                nc.vector.tensor_copy(
                    invb2.rearrange("p (a b) -> p a b", a=8),
                    invf.rearrange("p (a o) -> p a o", o=1).broadcast_to([128, 8, 2]),
                )

                # itps2[2p+h, r] = inv_p[r]; mi = masked rows; misb = pos-scale rows
                itps2 = itpsp.tile([16, 128], dt.float32, tag="itps2")
                nc.tensor.matmul(itps2[:], invb2[:], identb[:], start=True, stop=True)
                mi = sb.tile([16, 128], dt.bfloat16, tag="mi")
                nc.vector.tensor_mul(mi[:], itps2[:], maskab[:])
                isb = sb.tile([16, 16], dt.bfloat16, tag="isb")
                nc.scalar.copy(isb[:], itps2[:, 0:16])
                misb = sb.tile([16, 256], dt.bfloat16, tag="misb")
                nc.vector.tensor_mul(
                    misb.rearrange("p (a b) -> p a b", a=16),
                    isb.rearrange("p (o b) -> p o b", o=1).broadcast_to([16, 16, 16]),
                    pattb.rearrange("p (a b) -> p a b", a=16),
                )

                # bcig[32*(j%4)+u, j//4, r] = inv_j[u]*inv_j[r]*mask_{h(u)}[r]
                # for u<16, exactly 0 for u>=16 (garbage rows -> exp(0)=1)
                bcps = bcpsp.tile([128, 2, 128], dt.float32, tag="bcps")
                for j in range(8):
                    jp, e = j % 4, j // 4
                    tp = {"tile_position": (0, 96)} if jp == 3 else {}
                    nc.tensor.matmul(
                        bcps[32 * jp : 32 * jp + 32, e, :],
                        misb[:, 32 * j : 32 * j + 32],
                        mi[:],
                        start=True, stop=True, **tp,
                    )
                bcig = sb.tile([128, 2, 128], dt.bfloat16, tag="bcig")
                nc.scalar.copy(bcig[:], bcps[:])

                # fully-scaled logits; excluded entries are exactly 0
                t2g = sb.tile([128, 2, 128], dt.float16, tag="t2g")
                nc.vector.tensor_mul(t2g[:], posG[:], bcig[:])

                for e in range(2):
                    escr = scrp.tile([128, 128], dt.float16, tag="escr")
                    nc.scalar.activation(
                        escr[:], t2g[:, e, :], AF.Exp,
                        accum_out=sums_stage[:, 2 * g + e : 2 * g + e + 1],
                    )
                l0scr = scrp.tile([128, 2, 2], dt.float16, tag="l0scr")
                nc.vector.tensor_mul(
                    l0scr[:],
                    t2g[:, :, 16:18],
                    l0mask.rearrange("p (o b) -> p o b", o=1).broadcast_to([128, 2, 2]),
                )
                nc.vector.reduce_sum(
                    l0_stage[:, 2 * g : 2 * g + 2], l0scr[:],
                    axis=mybir.AxisListType.X,
                )

            loop_cm = tc.For_i(0, loop_n, 1) if loop_n else nullcontext()
            with loop_cm:
                pending = None
                for g in range(N_GROUPS):
                    state = phase_a(g, mid=pending if stage >= 3 else None)
                    pending = (g, state)
                if stage >= 3:
                    phase_b(pending[0], *pending[1])

            final = stg.tile([128, 2], dt.float32, tag="final")
            if stage < 3:
                nc.vector.memset(final[:], 0.0)
                nc.vector.memset(sums_stage[:], 0.0)
                nc.vector.memset(l0_stage[:], 0.0)
            lnscr = stg.tile([128, 2 * N_GROUPS], dt.float32, tag="lnscr")
            nc.scalar.activation(
                lnscr[:], sums_stage[:], AF.Ln, bias=neg72[:],
                accum_out=final[:, 0:1],
            )
            nc.vector.reduce_sum(final[:, 1:2], l0_stage[:], axis=mybir.AxisListType.X)
            nc.gpsimd.dma_start(out=out_d[:], in_=final[:])

    nc.compile()
    return nc


def _consts():
    import ml_dtypes

    bf = ml_dtypes.bfloat16
    identb = np.eye(128, dtype=bf)
    # row r = 2p+h: mask_h over staged cols [pos16, origA, origB, negA55, negB55]
    maskab = np.zeros((16, 128), dtype=np.float32)
    maskab[0::2, 16] = 1.0
    maskab[0::2, 18:73] = 1.0
    maskab[1::2, 17] = 1.0
    maskab[1::2, 73:128] = 1.0
    # pattb[k, 32j+m]: nonzero iff m<16 and k == 2j + (0 if m<8 else 1)
    pattb = np.zeros((16, 256), dtype=np.float32)
    for j in range(8):
        for m in range(16):
            k = 2 * j + (0 if m < 8 else 1)
            pattb[k, 32 * j + m] = 1.0
    # partition p = 32*q' + u: valid pos rows are u<16; cp col is 16+h(u)
    l0mask = np.zeros((128, 2), dtype=np.float32)
    for p in range(128):
        u = p % 32
        if u < 16:
            l0mask[p, 0 if u < 8 else 1] = 1.0
    neg72 = np.full((128, 1), -72.0, dtype=np.float32)
    return identb, maskab.astype(bf), pattb.astype(bf), l0mask.astype(bf), neg72


_PERM_AB = np.array([0] * 8 + [1] * 8 + [0, 1] + [0] * 55 + [1] * 55)
_PERM_ROW = np.array(
    list(range(1, 9)) + list(range(1, 9)) + [0, 0]
    + list(range(9, 64)) + list(range(9, 64))
)


def _stage_core(xc, xdt):
    """xc: [8192, 1024] fp32 -> staged [8, 128, 8192] in xdt."""
    xb = xc.astype(xdt)                      # cast first (cheaper moves)
    xp = xb.reshape(64, 2, 64, D)[:, _PERM_AB, _PERM_ROW, :]   # [pair, r, d]
    xt = xp.reshape(64, 128, 8, 128).transpose(0, 3, 2, 1)     # [pair, dp, c, r]
    xt = xt.reshape(8, 8, 128, 8 * 128).transpose(0, 2, 1, 3)  # [g, dp, j, c*r]
    return np.ascontiguousarray(xt.reshape(8, 128, 8 * D))


def kernel(x, labels=None, fp8=True, **_unused):
    from concourse.bass_utils import run_bass_kernel_spmd
    import concourse.mybir as mybir

    x = np.ascontiguousarray(np.asarray(x, dtype=np.float32))
    assert x.shape == (N_CORES * ROWS_PER_CORE, D), x.shape

    key = "fp8" if fp8 else "bf16"
    if key not in _CACHE:
        _CACHE[key] = _build(fp8=fp8)
    nc = _CACHE[key]

    xdt = mybir.dt.np(mybir.dt.float8e4) if fp8 else None
    if xdt is None:
        import ml_dtypes

        xdt = ml_dtypes.bfloat16

    identb, maskab, pattb, l0mask, neg72 = _consts()
    in_maps = [
        {
            "x": _stage_core(x[i * ROWS_PER_CORE : (i + 1) * ROWS_PER_CORE], xdt),
            "identb": identb,
            "maskab": maskab,
            "pattb": pattb,
            "l0mask": l0mask,
            "neg72": neg72,
        }
        for i in range(N_CORES)
    ]
    res = run_bass_kernel_spmd(nc, in_maps, list(range(N_CORES)))

    valid = (np.arange(128) % 32) < 16
    total = 0.0
    for r in res.results:
        o = r["out"].astype(np.float64)
        total += o[valid, 0].sum() - o[valid, 1].sum()
    loss = total / (1024.0 * NPOS)
    return np.array(loss, dtype=np.float32)
